# revision 13
# baseline (speedup 1.0000x reference)
"""Trainium2 Bass kernel for nn_Decoder (LSTM decoder w/ attention + vocab proj).

Sharding: data-parallel over batch. B=32 sequences are split 4-per-core across
8 NeuronCores; each core runs the full pipeline (embedding-matmul, LSTM
recurrence, attention, vocab projection) for its 4 sequences and writes its
[4, T, V] logits slice plus final h/c. No collectives.

Layouts (suffix T = feature-major / transposed):
- flattened (b, t) index r = t*BL + b (t-major)
- LSTM gate columns are host-permuted to H-chunk-interleaved order:
  chunk q (512 cols) = [i_q | f_q | o_q | g_q], each 128 wide, for H dims
  q*128..(q+1)*128. This lets the per-chunk pointwise chain start as soon
  as chunk q's matmuls finish, pipelined under later chunks' matmuls.
"""

import sys

import numpy as np

if "/opt/trn_rl_repo" not in sys.path:
    sys.path.insert(0, "/opt/trn_rl_repo")

import ml_dtypes  # noqa: E402
import concourse.bass as bass  # noqa: E402
import concourse.mybir as mybir  # noqa: E402
import concourse.tile as tile  # noqa: E402
from concourse import bacc  # noqa: E402
from concourse.bass_utils import run_bass_kernel_spmd  # noqa: E402
from concourse.masks import make_identity  # noqa: E402

F32 = mybir.dt.float32
F32R = mybir.dt.float32r
BF16 = mybir.dt.bfloat16
AF = mybir.ActivationFunctionType
ALU = mybir.AluOpType

B, T, S, H, E, V = 32, 64, 64, 512, 512, 32000
NH = 4 * H
H2 = 2 * H
NCORES = 8
BL = B // NCORES

BF16_NP = ml_dtypes.bfloat16


def gate_perm():
    """Permutation of the 4H gate axis into H-chunk-interleaved [i|f|o|g] order."""
    order = []
    for q in range(4):
        for base in (0, H, 3 * H, 2 * H):  # i, f, o, g
            order.append(np.arange(base + q * 128, base + q * 128 + 128))
    return np.concatenate(order)


def build_decoder(T=T, V=V, BL=BL):
    nc = bacc.Bacc(None, target_bir_lowering=False)

    BT = BL * T
    chunks = []
    off = 0
    while off < BT:
        sz = min(128, BT - off)
        chunks.append((off, sz))
        off += sz
    nsl = []
    off = 0
    while off < V:
        w = min(512, V - off)
        nsl.append((off, w))
        off += w

    # ---- DRAM I/O ----
    xt_d = nc.dram_tensor("xt", [E, BT], F32, kind="ExternalInput")
    wih_d = nc.dram_tensor("wih_t", [E, NH], F32, kind="ExternalInput")
    whh_d = nc.dram_tensor("whh_t", [H, NH], F32, kind="ExternalInput")
    bih_d = nc.dram_tensor("b_ih", [NH], F32, kind="ExternalInput")
    bhh_d = nc.dram_tensor("b_hh", [NH], F32, kind="ExternalInput")
    encl_d = nc.dram_tensor("enc_l", [S, BL * H2], BF16, kind="ExternalInput")
    enct_d = nc.dram_tensor("enc_t", [H2, BL * S], BF16, kind="ExternalInput")
    watt_d = nc.dram_tensor("watt_t", [H, H2], BF16, kind="ExternalInput")
    wcomb_d = nc.dram_tensor("wcomb_t", [3 * H, H], BF16, kind="ExternalInput")
    bcomb_d = nc.dram_tensor("b_comb", [H], F32, kind="ExternalInput")
    wo_d = nc.dram_tensor("wo_t", [H, V], BF16, kind="ExternalInput")
    bo_d = nc.dram_tensor("b_out", [V], F32, kind="ExternalInput")
    h0t_d = nc.dram_tensor("h0t", [128, 4 * BL], F32, kind="ExternalInput")
    c0_d = nc.dram_tensor("c0", [BL, H], F32, kind="ExternalInput")

    lg_d = nc.dram_tensor("logits", [BL, T, V], F32, kind="ExternalOutput")
    ho_d = nc.dram_tensor("h_out", [BL, H], F32, kind="ExternalOutput")
    co_d = nc.dram_tensor("c_out", [BL, H], F32, kind="ExternalOutput")
    xw_d = nc.dram_tensor("xw_scratch", [BT, NH], F32)

    wo_view = wo_d.rearrange("(k p) v -> p k v", p=128)

    def lg_ap(t0, tcs, noff, nw):
        return bass.AP(lg_d, t0 * V + noff, [[V, tcs], [T * V, BL], [1, nw]])

    with tile.TileContext(nc) as tc:
        with (
            tc.tile_pool(name="const", bufs=1) as constp,
            tc.tile_pool(name="wts", bufs=1) as wts,
            tc.tile_pool(name="persist", bufs=1) as pers,
            tc.tile_pool(name="stg", bufs=1) as stgp,
            tc.tile_pool(name="xwq", bufs=2) as xwqp,
            tc.tile_pool(name="pw", bufs=2) as pw,
            tc.tile_pool(name="hTk", bufs=2) as hTkp,
            tc.tile_pool(name="attn", bufs=1) as attnp,
            tc.tile_pool(name="wo", bufs=3) as wop,
            tc.tile_pool(name="lgs", bufs=2) as lgsp,
            tc.tile_pool(name="bo", bufs=1) as bop,
            tc.tile_pool(name="ps_g", bufs=2, space="PSUM") as psg,
            tc.tile_pool(name="ps_tr", bufs=1, space="PSUM") as pstr,
            tc.tile_pool(name="ps_at", bufs=2, space="PSUM") as psat,
            tc.tile_pool(name="ps_lg", bufs=2, space="PSUM") as pslg,
        ):
            # ---------- constants & weights ----------
            ident = constp.tile([128, 128], F32, tag="ident")
            make_identity(nc, ident[:])
            ones_f = constp.tile([1, 128], F32, tag="ones_f")
            nc.vector.memset(ones_f[:], 1.0)
            ones = constp.tile([1, 128], F32R, tag="ones")
            nc.vector.tensor_copy(ones[:], ones_f[:])

            whh_sb = []
            for k in range(4):
                t_ = wts.tile([128, NH], F32R, tag=f"whh{k}")
                for hh in range(2):
                    stg = stgp.tile([128, NH // 2], F32, tag="stg")
                    nc.sync.dma_start(
                        stg[:], whh_d[k * 128:(k + 1) * 128,
                                      hh * (NH // 2):(hh + 1) * (NH // 2)]
                    )
                    nc.vector.tensor_copy(
                        t_[:, hh * (NH // 2):(hh + 1) * (NH // 2)], stg[:]
                    )
                whh_sb.append(t_)
            watt_sb = []
            for k in range(4):
                t_ = wts.tile([128, H2], BF16, tag=f"watt{k}")
                nc.sync.dma_start(t_[:], watt_d[k * 128:(k + 1) * 128, :])
                watt_sb.append(t_)
            wcomb_sb = []
            for k in range(12):
                t_ = wts.tile([128, H], BF16, tag=f"wcomb{k}")
                nc.sync.dma_start(t_[:], wcomb_d[k * 128:(k + 1) * 128, :])
                wcomb_sb.append(t_)
            encl_sb = wts.tile([S, BL * H2], BF16, tag="encl")
            nc.sync.dma_start(encl_sb[:], encl_d[:, :])
            enct_sb = []
            for k in range(8):
                t_ = wts.tile([128, BL * S], BF16, tag=f"enct{k}")
                nc.sync.dma_start(t_[:], enct_d[k * 128:(k + 1) * 128, :])
                enct_sb.append(t_)
            bcomb_sb = constp.tile([128, 4], F32, tag="bcomb")
            nc.sync.dma_start(bcomb_sb[:], bcomb_d.rearrange("(m p) -> p m", p=128))

            bih_sb = constp.tile([1, NH], F32, tag="bih")
            nc.sync.dma_start(bih_sb[:], bih_d.rearrange("(o n) -> o n", o=1))
            bhh_sb = constp.tile([1, NH], F32, tag="bhh")
            nc.sync.dma_start(bhh_sb[:], bhh_d.rearrange("(o n) -> o n", o=1))
            bihh = constp.tile([1, NH], F32R, tag="bihh")
            nc.vector.tensor_add(bihh[:], bih_sb[:], bhh_sb[:])

            xt_sb = []
            for k in range(4):
                stg = stgp.tile([128, NH // 2], F32, tag="stg")
                nc.sync.dma_start(stg[:, :BT], xt_d[k * 128:(k + 1) * 128, :])
                t_ = wts.tile([128, BT], F32R, tag=f"xt{k}")
                nc.vector.tensor_copy(t_[:], stg[:, :BT])
                xt_sb.append(t_)

            # persistent stores, col layout (k-chunk, t*BL+b)
            hTb = pers.tile([128, 4 * BT], BF16, tag="hTb")
            outsT = pers.tile([128, 4 * BT], BF16, tag="outsT")

            # ---------- xW = X @ W_ih.T + (b_ih+b_hh), gate-permuted -> DRAM ----------
            for n4 in range(4):
                wih_t = []
                for k in range(4):
                    stg = stgp.tile([128, NH // 2], F32, tag="stg")
                    nc.sync.dma_start(
                        stg[:, :512],
                        wih_d[k * 128:(k + 1) * 128, n4 * 512:(n4 + 1) * 512],
                    )
                    w_ = wop.tile([128, NH], F32R, tag="wo", name=f"wihs{n4}_{k}")
                    nc.vector.tensor_copy(w_[:, :512], stg[:, :512])
                    wih_t.append(w_)
                for (moff, msz) in chunks:
                    ps = psg.tile([128, 512], F32, tag="g")
                    for k in range(4):
                        nc.tensor.matmul(
                            ps[:msz, :],
                            xt_sb[k][:, moff:moff + msz],
                            wih_t[k][:, :512],
                            start=(k == 0), stop=False,
                        )
                    nc.tensor.matmul(
                        ps[:msz, :], ones[:1, :msz],
                        bihh[:1, n4 * 512:(n4 + 1) * 512],
                        start=False, stop=True,
                    )
                    xws = stgp.tile([128, NH // 2], F32, tag="stg", name=f"xwev{n4}_{moff}")
                    nc.vector.tensor_copy(xws[:msz, :512], ps[:msz, :])
                    nc.sync.dma_start(
                        xw_d[moff:moff + msz, n4 * 512:(n4 + 1) * 512], xws[:msz, :512]
                    )

            # ---------- initial state ----------
            hTk = [None] * 4
            stg0 = stgp.tile([128, NH // 2], F32, tag="stg")
            nc.sync.dma_start(stg0[:, :4 * BL], h0t_d[:, :])
            for k in range(4):
                t_ = hTkp.tile([128, BL], F32R, tag=f"hT{k}", name=f"hT0_{k}")
                nc.vector.tensor_copy(t_[:], stg0[:, k * BL:(k + 1) * BL])
                hTk[k] = t_
            c_cur = [None] * 4
            for q in range(4):
                t_ = pw.tile([BL, 128], F32, tag=f"c{q}", name=f"c0_{q}")
                nc.sync.dma_start(t_[:], c0_d[:, q * 128:(q + 1) * 128])
                c_cur[q] = t_

            # ---------- attention + vocab for one bt-chunk ----------
            def attn_and_vocab(m, moff, msz):
                tcs = msz // BL
                qt_sb = []
                for k8 in range(8):
                    ps = psat.tile([128, 128], F32, tag="at")
                    for k in range(4):
                        nc.tensor.matmul(
                            ps[:, :msz],
                            watt_sb[k][:, k8 * 128:(k8 + 1) * 128],
                            hTb[:, k * BT + moff:k * BT + moff + msz],
                            start=(k == 0), stop=(k == 3),
                        )
                    q_ = attnp.tile([128, 128], BF16, tag=f"qt{k8}")
                    nc.vector.tensor_copy(q_[:, :msz], ps[:, :msz])
                    qt_sb.append(q_)
                wt_sb = []
                for b in range(BL):
                    ps = psat.tile([128, 128], F32, tag="at")
                    for k8 in range(8):
                        qcols = qt_sb[k8].rearrange("p (t b) -> p t b", b=BL)[:, :tcs, b]
                        nc.tensor.matmul(
                            ps[:S, :tcs],
                            enct_sb[k8][:, b * S:(b + 1) * S],
                            qcols,
                            start=(k8 == 0), stop=(k8 == 7),
                        )
                    w_ = attnp.tile([S, 32], BF16, tag=f"wt{b}")
                    nc.vector.tensor_copy(w_[:, :tcs], ps[:S, :tcs])
                    wt_sb.append(w_)
                app_sb = [attnp.tile([128, 128], BF16, tag=f"app{m8}", name=f"app{m8}")
                          for m8 in range(8)]
                for b in range(BL):
                    for m8 in range(8):
                        ps = psat.tile([128, 128], F32, tag="at")
                        nc.tensor.matmul(
                            ps[:, :tcs],
                            encl_sb[:, b * H2 + m8 * 128: b * H2 + (m8 + 1) * 128],
                            wt_sb[b][:, :tcs],
                            start=True, stop=True,
                        )
                        app_cols = app_sb[m8].rearrange(
                            "p (t b) -> p t b", b=BL)[:, :tcs, b]
                        nc.vector.tensor_copy(app_cols, ps[:, :tcs])
                for m4 in range(4):
                    ps = psat.tile([128, 128], F32, tag="at")
                    for kk in range(12):
                        rhs = (hTb[:, kk * BT + moff:kk * BT + moff + msz] if kk < 4
                               else app_sb[kk - 4][:, :msz])
                        nc.tensor.matmul(
                            ps[:, :msz],
                            wcomb_sb[kk][:, m4 * 128:(m4 + 1) * 128],
                            rhs,
                            start=(kk == 0), stop=(kk == 11),
                        )
                    nc.scalar.activation(
                        outsT[:, m4 * BT + moff:m4 * BT + moff + msz], ps[:, :msz],
                        AF.Tanh, bias=bcomb_sb[:, m4:m4 + 1],
                    )
                for (noff, nw) in nsl:
                    wo_sb = wop.tile([128, NH], BF16, tag="wo")
                    nc.sync.dma_start(
                        wo_sb.rearrange("p (k v) -> p k v", k=4)[:, :, :nw],
                        wo_view[:, :, noff:noff + nw],
                    )
                    bo1 = bop.tile([1, 512], F32, tag="bo1")
                    nc.scalar.dma_start(
                        bo1[:, :nw],
                        bo_d.rearrange("(o n) -> o n", o=1)[:, noff:noff + nw],
                    )
                    bob = bop.tile([128, 512], F32, tag="bob")
                    nc.gpsimd.partition_broadcast(bob[:, :nw], bo1[:1, :nw])
                    ps = pslg.tile([128, 512], F32, tag="lg")
                    for k in range(4):
                        nc.tensor.matmul(
                            ps[:msz, :nw],
                            outsT[:, k * BT + moff:k * BT + moff + msz],
                            wo_sb[:, k * 512:k * 512 + nw],
                            start=(k == 0), stop=(k == 3),
                        )
                    lgs = lgsp.tile([128, 512], F32, tag="lgs")
                    nc.vector.tensor_tensor(
                        lgs[:msz, :nw], ps[:msz, :nw], bob[:msz, :nw], ALU.add
                    )
                    nc.gpsimd.dma_start(
                        lg_ap(moff // BL, tcs, noff, nw), lgs[:msz, :nw]
                    )

            # ---------- LSTM recurrence ----------
            for t in range(T):
                xws = xwqp.tile([BL, NH], F32, tag="xwt")
                nc.scalar.dma_start(xws[:], xw_d[t * BL:(t + 1) * BL, :])
                hTk_next = [None] * 4
                c_next = [None] * 4
                for q in range(4):
                    ps = psg.tile([128, 512], F32, tag="g")
                    for k in range(4):
                        nc.tensor.matmul(
                            ps[:BL, :],
                            hTk[k][:],
                            whh_sb[k][:, q * 512:(q + 1) * 512],
                            start=(k == 0), stop=(k == 3),
                        )
                    g_ = pw.tile([BL, 512], F32, tag=f"g{q}")
                    nc.vector.tensor_tensor(
                        g_[:], ps[:BL, :], xws[:, q * 512:(q + 1) * 512], ALU.add
                    )
                    act = pw.tile([BL, 512], F32, tag=f"a{q}")
                    nc.scalar.activation(act[:, :384], g_[:, :384], AF.Sigmoid)
                    nc.scalar.activation(act[:, 384:512], g_[:, 384:512], AF.Tanh)
                    fc = pw.tile([BL, 128], F32, tag=f"fc{q}")
                    nc.gpsimd.tensor_tensor(
                        fc[:], act[:, 128:256], c_cur[q][:], ALU.mult
                    )
                    ig = pw.tile([BL, 128], F32, tag=f"ig{q}")
                    nc.vector.tensor_tensor(
                        ig[:], act[:, 0:128], act[:, 384:512], ALU.mult
                    )
                    cn = pw.tile([BL, 128], F32, tag=f"c{q}")
                    nc.gpsimd.tensor_tensor(cn[:], fc[:], ig[:], ALU.add)
                    tcq = pw.tile([BL, 128], F32, tag=f"tc{q}")
                    nc.scalar.activation(tcq[:], cn[:], AF.Tanh)
                    hq = pw.tile([BL, 128], F32, tag=f"h{q}")
                    nc.vector.tensor_tensor(hq[:], act[:, 256:384], tcq[:], ALU.mult)
                    pst = pstr.tile([128, BL], F32, tag=f"tr{q % 2}")
                    nc.tensor.transpose(pst[:, :], hq[:], ident[:BL, :BL])
                    hTn = hTkp.tile([128, BL], F32R, tag=f"hT{q}")
                    nc.vector.tensor_copy(hTn[:], pst[:, :])
                    nc.scalar.activation(
                        hTb[:, q * BT + t * BL:q * BT + (t + 1) * BL], pst[:, :],
                        AF.Copy,
                    )
                    hTk_next[q] = hTn
                    c_next[q] = cn
                    if t == T - 1:
                        nc.sync.dma_start(ho_d[:, q * 128:(q + 1) * 128], hq[:])
                        nc.sync.dma_start(co_d[:, q * 128:(q + 1) * 128], cn[:])
                hTk = hTk_next
                c_cur = c_next

                for m, (moff, msz) in enumerate(chunks):
                    if t == (moff + msz) // BL - 1:
                        attn_and_vocab(m, moff, msz)

    nc.compile()
    return nc


_NC_CACHE = {}


def _get_nc(T_=T, V_=V, BL_=BL):
    key = (T_, V_, BL_)
    if key not in _NC_CACHE:
        _NC_CACHE[key] = build_decoder(T_, V_, BL_)
    return _NC_CACHE[key]


def _prep_core(c, x_all, enc, h0, c0, common):
    sl = slice(c * BL, (c + 1) * BL)
    xs = np.ascontiguousarray(x_all[sl])
    e_ = np.ascontiguousarray(enc[sl])
    h0s = np.ascontiguousarray(h0[sl])
    c0s = np.ascontiguousarray(c0[sl])
    d = dict(common)
    d["xt"] = xs.transpose(2, 1, 0).reshape(E, T * BL).astype(np.float32)
    d["enc_l"] = e_.transpose(1, 0, 2).reshape(S, BL * H2).astype(BF16_NP)
    d["enc_t"] = e_.transpose(2, 0, 1).reshape(H2, BL * S).astype(BF16_NP)
    d["h0t"] = (
        h0s.T.reshape(4, 128, BL).transpose(1, 0, 2).reshape(128, 4 * BL)
        .astype(np.float32)
    )
    d["c0"] = c0s.astype(np.float32)
    return d


def kernel(inputs, encoder_outputs, h0, c0, emb, W_ih, W_hh, b_ih, b_hh,
           W_att, W_comb, b_comb, W_out, b_out):
    inputs = np.asarray(inputs)
    enc = np.asarray(encoder_outputs, dtype=np.float32)
    h0 = np.asarray(h0, dtype=np.float32)
    c0 = np.asarray(c0, dtype=np.float32)
    emb = np.asarray(emb, dtype=np.float32)
    W_ih = np.asarray(W_ih, dtype=np.float32)
    W_hh = np.asarray(W_hh, dtype=np.float32)
    b_ih = np.asarray(b_ih, dtype=np.float32)
    b_hh = np.asarray(b_hh, dtype=np.float32)
    W_att = np.asarray(W_att, dtype=np.float32)
    W_comb = np.asarray(W_comb, dtype=np.float32)
    b_comb = np.asarray(b_comb, dtype=np.float32)
    W_out = np.asarray(W_out, dtype=np.float32)
    b_out = np.asarray(b_out, dtype=np.float32)

    x_all = emb[inputs.astype(np.int64)]  # embedding gather (sharding prep)
    perm = gate_perm()

    common = {
        "wih_t": np.ascontiguousarray(W_ih.T[:, perm]).astype(np.float32),
        "whh_t": np.ascontiguousarray(W_hh.T[:, perm]).astype(np.float32),
        "b_ih": np.ascontiguousarray(b_ih[perm]),
        "b_hh": np.ascontiguousarray(b_hh[perm]),
        "watt_t": np.ascontiguousarray(W_att.T).astype(BF16_NP),
        "wcomb_t": np.ascontiguousarray(W_comb.T).astype(BF16_NP),
        "b_comb": b_comb,
        "wo_t": np.ascontiguousarray(W_out.T).astype(BF16_NP),
        "b_out": b_out,
    }
    in_maps = [_prep_core(c, x_all, enc, h0, c0, common) for c in range(NCORES)]

    nc = _get_nc()
    res = run_bass_kernel_spmd(nc, in_maps, core_ids=list(range(NCORES)))
    global LAST_RESULT
    LAST_RESULT = res
    logits = np.concatenate([r["logits"] for r in res.results], axis=0)
    h = np.concatenate([r["h_out"] for r in res.results], axis=0)
    c = np.concatenate([r["c_out"] for r in res.results], axis=0)
    return logits, h, c


# revision 18
# speedup vs baseline: 1.4418x; 1.4418x over previous
"""Trainium2 Bass kernel for nn_Decoder (LSTM decoder w/ attention + vocab proj).

Sharding: data-parallel over batch. B=32 sequences are split 4-per-core across
8 NeuronCores; each core runs the full pipeline (embedding-matmul, LSTM
recurrence, attention, vocab projection) for its 4 sequences and writes its
[4, T, V] logits slice plus final h/c. No collectives.

Layouts (suffix T = feature-major / transposed):
- flattened (b, t) index r = t*BL + b (t-major)
- LSTM gate columns are host-permuted to H-chunk-interleaved order:
  chunk q (512 cols) = [i_q | f_q | o_q | g_q], each 128 wide, for H dims
  q*128..(q+1)*128. This lets the per-chunk pointwise chain start as soon
  as chunk q's matmuls finish, pipelined under later chunks' matmuls.
"""

import sys

import numpy as np

if "/opt/trn_rl_repo" not in sys.path:
    sys.path.insert(0, "/opt/trn_rl_repo")

import ml_dtypes  # noqa: E402
import concourse.bass as bass  # noqa: E402
import concourse.mybir as mybir  # noqa: E402
import concourse.tile as tile  # noqa: E402
from concourse import bacc  # noqa: E402
from concourse.bass_utils import run_bass_kernel_spmd  # noqa: E402
from concourse.masks import make_identity  # noqa: E402

F32 = mybir.dt.float32
F32R = mybir.dt.float32r
BF16 = mybir.dt.bfloat16
AF = mybir.ActivationFunctionType
ALU = mybir.AluOpType

B, T, S, H, E, V = 32, 64, 64, 512, 512, 32000
NH = 4 * H
H2 = 2 * H
NCORES = 8
BL = B // NCORES

BF16_NP = ml_dtypes.bfloat16


def gate_perm():
    """Permutation of the 4H gate axis into H-chunk-interleaved [i|f|o|g] order."""
    order = []
    for q in range(4):
        for base in (0, H, 3 * H, 2 * H):  # i, f, o, g
            order.append(np.arange(base + q * 128, base + q * 128 + 128))
    return np.concatenate(order)


def build_decoder(T=T, V=V, BL=BL):
    nc = bacc.Bacc(None, target_bir_lowering=False)

    BT = BL * T
    chunks = []
    off = 0
    while off < BT:
        sz = min(128, BT - off)
        chunks.append((off, sz))
        off += sz
    nsl = []
    off = 0
    while off < V:
        w = min(512, V - off)
        nsl.append((off, w))
        off += w

    # ---- DRAM I/O ----
    xt_d = nc.dram_tensor("xt", [E, BT], F32, kind="ExternalInput")
    wih_d = nc.dram_tensor("wih_t", [E, NH], F32, kind="ExternalInput")
    whh_d = nc.dram_tensor("whh_t", [H, NH], F32, kind="ExternalInput")
    bih_d = nc.dram_tensor("b_ih", [NH], F32, kind="ExternalInput")
    bhh_d = nc.dram_tensor("b_hh", [NH], F32, kind="ExternalInput")
    encl_d = nc.dram_tensor("enc_l", [S, BL * H2], BF16, kind="ExternalInput")
    enct_d = nc.dram_tensor("enc_t", [H2, BL * S], BF16, kind="ExternalInput")
    watt_d = nc.dram_tensor("watt_t", [H, H2], BF16, kind="ExternalInput")
    wcomb_d = nc.dram_tensor("wcomb_t", [3 * H, H], BF16, kind="ExternalInput")
    bcomb_d = nc.dram_tensor("b_comb", [H], F32, kind="ExternalInput")
    wo_d = nc.dram_tensor("wo_t", [H, V], BF16, kind="ExternalInput")
    bo_d = nc.dram_tensor("b_out", [V], F32, kind="ExternalInput")
    h0t_d = nc.dram_tensor("h0t", [128, 4 * BL], F32, kind="ExternalInput")
    c0_d = nc.dram_tensor("c0", [BL, H], F32, kind="ExternalInput")

    lg_d = nc.dram_tensor("logits", [BL, T, V], F32, kind="ExternalOutput")
    ho_d = nc.dram_tensor("h_out", [BL, H], F32, kind="ExternalOutput")
    co_d = nc.dram_tensor("c_out", [BL, H], F32, kind="ExternalOutput")

    wo_view = wo_d.rearrange("(k p) v -> p k v", p=128)

    def lg_ap(t0, tcs, noff, nw):
        return bass.AP(lg_d, t0 * V + noff, [[V, tcs], [T * V, BL], [1, nw]])

    with tile.TileContext(nc) as tc:
        with (
            tc.tile_pool(name="const", bufs=1) as constp,
            tc.tile_pool(name="wts", bufs=1) as wts,
            tc.tile_pool(name="persist", bufs=1) as pers,
            tc.tile_pool(name="stg", bufs=1) as stgp,
            tc.tile_pool(name="pw", bufs=2) as pw,
            tc.tile_pool(name="hTk", bufs=2) as hTkp,
            tc.tile_pool(name="attn", bufs=1) as attnp,
            tc.tile_pool(name="wo", bufs=3) as wop,
            tc.tile_pool(name="lgs", bufs=3) as lgsp,
            tc.tile_pool(name="bo", bufs=1) as bop,
            tc.tile_pool(name="ps_g", bufs=1, space="PSUM") as psg,
            tc.tile_pool(name="ps_tr", bufs=1, space="PSUM") as pstr,
            tc.tile_pool(name="ps_at", bufs=2, space="PSUM") as psat,
        ):
            # ---------- constants & weights ----------
            ident = constp.tile([128, 128], F32, tag="ident")
            make_identity(nc, ident[:])
            ones_f = constp.tile([1, 128], F32, tag="ones_f")
            nc.vector.memset(ones_f[:], 1.0)
            ones = constp.tile([1, 128], F32R, tag="ones")
            nc.vector.tensor_copy(ones[:], ones_f[:])
            ident_r = constp.tile([128, 128], F32R, tag="ident_r")
            nc.vector.tensor_copy(ident_r[:], ident[:])
            ones_b = constp.tile([1, 128], BF16, tag="ones_b")
            nc.vector.tensor_copy(ones_b[:], ones_f[:])

            whh_sb = []
            for k in range(4):
                t_ = wts.tile([128, NH], F32R, tag=f"whh{k}")
                for hh in range(2):
                    stg = stgp.tile([128, NH // 2], F32, tag="stg")
                    nc.sync.dma_start(
                        stg[:], whh_d[k * 128:(k + 1) * 128,
                                      hh * (NH // 2):(hh + 1) * (NH // 2)]
                    )
                    nc.vector.tensor_copy(
                        t_[:, hh * (NH // 2):(hh + 1) * (NH // 2)], stg[:]
                    )
                whh_sb.append(t_)
            watt_sb = []
            for k in range(4):
                t_ = wts.tile([128, H2], BF16, tag=f"watt{k}")
                nc.sync.dma_start(t_[:], watt_d[k * 128:(k + 1) * 128, :])
                watt_sb.append(t_)
            wcomb_sb = []
            for k in range(12):
                t_ = wts.tile([128, H], BF16, tag=f"wcomb{k}")
                nc.sync.dma_start(t_[:], wcomb_d[k * 128:(k + 1) * 128, :])
                wcomb_sb.append(t_)
            encl_sb = wts.tile([S, BL * H2], BF16, tag="encl")
            nc.sync.dma_start(encl_sb[:], encl_d[:, :])
            enct_sb = []
            for k in range(8):
                t_ = wts.tile([128, BL * S], BF16, tag=f"enct{k}")
                nc.sync.dma_start(t_[:], enct_d[k * 128:(k + 1) * 128, :])
                enct_sb.append(t_)
            bcomb_sb = constp.tile([128, 4], F32, tag="bcomb")
            nc.sync.dma_start(bcomb_sb[:], bcomb_d.rearrange("(m p) -> p m", p=128))

            bih_sb = constp.tile([1, NH], F32, tag="bih")
            nc.sync.dma_start(bih_sb[:], bih_d.rearrange("(o n) -> o n", o=1))
            bhh_sb = constp.tile([1, NH], F32, tag="bhh")
            nc.sync.dma_start(bhh_sb[:], bhh_d.rearrange("(o n) -> o n", o=1))
            bihh = constp.tile([1, NH], F32R, tag="bihh")
            nc.vector.tensor_add(bihh[:], bih_sb[:], bhh_sb[:])

            xt_sb = []
            for k in range(4):
                stg = stgp.tile([128, NH // 2], F32, tag="stg")
                nc.sync.dma_start(stg[:, :BT], xt_d[k * 128:(k + 1) * 128, :])
                t_ = wts.tile([128, BT], F32R, tag=f"xt{k}")
                nc.vector.tensor_copy(t_[:], stg[:, :BT])
                xt_sb.append(t_)

            # persistent stores, per bt-chunk, col layout (k-chunk, local t*BL+b)
            hTb_m = [pers.tile([128, 4 * msz], BF16, tag=f"hTb{mi}", name=f"hTb{mi}")
                     for mi, (mo, msz) in enumerate(chunks)]
            outsT_m = [pers.tile([128, 4 * msz], BF16, tag=f"outsT{mi}",
                                 name=f"outsT{mi}")
                       for mi, (mo, msz) in enumerate(chunks)]
            xw_sb = []
            for mi, (mo_, msz_) in enumerate(chunks):
                t_ = wts.tile([128, NH], F32R, tag=f"xwsb{mi}", name=f"xwsb{mi}")
                if msz_ < 128:
                    zf = stgp.tile([128, NH // 2], F32, tag="stg", name=f"z{mi}")
                    nc.vector.memset(zf[:, :1], 0.0)
                    for hh in range(2):
                        nc.vector.tensor_copy(
                            t_[:, hh * (NH // 2):(hh + 1) * (NH // 2)],
                            zf[:, :1].broadcast_to([128, NH // 2]),
                        )
                xw_sb.append(t_)

            # ---------- xW = X @ W_ih.T + (b_ih+b_hh), gate-permuted -> SBUF ----------
            for n4 in range(4):
                for mi, (moff, msz) in enumerate(chunks):
                    ps = psg.tile([128, 512], F32, tag="g0")
                    for k in range(4):
                        stg = stgp.tile([128, NH // 2], F32, tag="stg")
                        nc.sync.dma_start(
                            stg[:, :512],
                            wih_d[k * 128:(k + 1) * 128, n4 * 512:(n4 + 1) * 512],
                        )
                        w_ = wop.tile([128, NH], F32R, tag="wo",
                                      name=f"wihs{n4}_{mi}_{k}")
                        nc.vector.tensor_copy(w_[:, :512], stg[:, :512])
                        nc.tensor.matmul(
                            ps[:msz, :],
                            xt_sb[k][:, moff:moff + msz],
                            w_[:, :512],
                            start=(k == 0), stop=False,
                        )
                    nc.tensor.matmul(
                        ps[:msz, :], ones[:1, :msz],
                        bihh[:1, n4 * 512:(n4 + 1) * 512],
                        start=False, stop=True,
                    )
                    nc.vector.tensor_copy(
                        xw_sb[mi][:msz, n4 * 512:(n4 + 1) * 512], ps[:msz, :]
                    )

            # ---------- initial state ----------
            hTk = [None] * 4
            stg0 = stgp.tile([128, NH // 2], F32, tag="stg")
            nc.sync.dma_start(stg0[:, :4 * BL], h0t_d[:, :])
            for k in range(4):
                t_ = hTkp.tile([128, BL], F32R, tag=f"hT{k}", name=f"hT0_{k}")
                nc.vector.tensor_copy(t_[:], stg0[:, k * BL:(k + 1) * BL])
                hTk[k] = t_
            c_cur = [None] * 4
            for q in range(4):
                t_ = pw.tile([BL, 128], F32, tag=f"c{q}", name=f"c0_{q}")
                nc.sync.dma_start(t_[:], c0_d[:, q * 128:(q + 1) * 128])
                c_cur[q] = t_

            # ---------- attention + vocab for one bt-chunk ----------
            def attn_and_vocab(m, moff, msz):
                tcs = msz // BL
                hTb = hTb_m[m]
                outsT = outsT_m[m]
                qt_sb = []
                for k8 in range(8):
                    ps = psat.tile([128, 512], F32, tag="atlg")
                    for k in range(4):
                        nc.tensor.matmul(
                            ps[:, :msz],
                            watt_sb[k][:, k8 * 128:(k8 + 1) * 128],
                            hTb[:, k * msz:(k + 1) * msz],
                            start=(k == 0), stop=(k == 3),
                        )
                    q_ = attnp.tile([128, 128], BF16, tag=f"qt{k8}")
                    nc.vector.tensor_copy(q_[:, :msz], ps[:, :msz])
                    qt_sb.append(q_)
                wt_sb = []
                for b in range(BL):
                    ps = psat.tile([128, 512], F32, tag="atlg")
                    for k8 in range(8):
                        qcols = qt_sb[k8].rearrange("p (t b) -> p t b", b=BL)[:, :tcs, b]
                        nc.tensor.matmul(
                            ps[:S, :tcs],
                            enct_sb[k8][:, b * S:(b + 1) * S],
                            qcols,
                            start=(k8 == 0), stop=(k8 == 7),
                        )
                    w_ = attnp.tile([S, 32], BF16, tag=f"wt{b}")
                    nc.vector.tensor_copy(w_[:, :tcs], ps[:S, :tcs])
                    wt_sb.append(w_)
                app_sb = [attnp.tile([128, 128], BF16, tag=f"app{m8}", name=f"app{m8}")
                          for m8 in range(8)]
                for b in range(BL):
                    for m8 in range(8):
                        ps = psat.tile([128, 512], F32, tag="atlg")
                        nc.tensor.matmul(
                            ps[:, :tcs],
                            encl_sb[:, b * H2 + m8 * 128: b * H2 + (m8 + 1) * 128],
                            wt_sb[b][:, :tcs],
                            start=True, stop=True,
                        )
                        app_cols = app_sb[m8].rearrange(
                            "p (t b) -> p t b", b=BL)[:, :tcs, b]
                        nc.vector.tensor_copy(app_cols, ps[:, :tcs])
                for m4 in range(4):
                    ps = psat.tile([128, 512], F32, tag="atlg")
                    for kk in range(12):
                        rhs = (hTb[:, kk * msz:(kk + 1) * msz] if kk < 4
                               else app_sb[kk - 4][:, :msz])
                        nc.tensor.matmul(
                            ps[:, :msz],
                            wcomb_sb[kk][:, m4 * 128:(m4 + 1) * 128],
                            rhs,
                            start=(kk == 0), stop=(kk == 11),
                        )
                    nc.scalar.activation(
                        outsT[:, m4 * msz:(m4 + 1) * msz], ps[:, :msz],
                        AF.Tanh, bias=bcomb_sb[:, m4:m4 + 1],
                    )
                for (noff, nw) in nsl:
                    wo_sb = wop.tile([128, NH], BF16, tag="wo")
                    nc.sync.dma_start(
                        wo_sb.rearrange("p (k v) -> p k v", k=4)[:, :, :nw],
                        wo_view[:, :, noff:noff + nw],
                    )
                    bo1f = bop.tile([1, 512], F32, tag="bo1f")
                    nc.scalar.dma_start(
                        bo1f[:, :nw],
                        bo_d.rearrange("(o n) -> o n", o=1)[:, noff:noff + nw],
                    )
                    bo1b = bop.tile([1, 512], BF16, tag="bo1b")
                    nc.vector.tensor_copy(bo1b[:, :nw], bo1f[:, :nw])
                    ps = psat.tile([128, 512], F32, tag="atlg")
                    nc.tensor.matmul(
                        ps[:msz, :nw], ones_b[:1, :msz], bo1b[:1, :nw],
                        start=True, stop=False,
                    )
                    for k in range(4):
                        nc.tensor.matmul(
                            ps[:msz, :nw],
                            outsT[:, k * msz:(k + 1) * msz],
                            wo_sb[:, k * 512:k * 512 + nw],
                            start=False, stop=(k == 3),
                        )
                    lgs = lgsp.tile([128, 512], F32, tag="lgs")
                    nc.scalar.activation(lgs[:msz, :nw], ps[:msz, :nw], AF.Copy)
                    nc.sync.dma_start(
                        lg_ap(moff // BL, tcs, noff, nw), lgs[:msz, :nw]
                    )

            # ---------- LSTM recurrence ----------
            # k-outer matmul groups; prev-step transposes interleaved per k so the
            # PE pipeline self-sustains across steps. xW folded in via identity-
            # column selector matmuls.
            prev_h = None

            def emit_tr(k, tprev):
                mi = (tprev * BL) // 128
                mo = chunks[mi][0]
                msz = chunks[mi][1]
                pst = pstr.tile([128, BL], F32, tag=f"tr{k % 2}")
                nc.tensor.transpose(pst[:, :], prev_h[k][:], ident[:BL, :BL])
                hTn = hTkp.tile([128, BL], F32R, tag=f"hT{k}")
                nc.vector.tensor_copy(hTn[:], pst[:, :])
                col = k * msz + tprev * BL - mo
                nc.scalar.activation(
                    hTb_m[mi][:, col:col + BL], pst[:, :], AF.Copy
                )
                return hTn

            for t in range(T):
                ps_g = [psg.tile([128, 512], F32, tag=f"g{q}", name=f"g{t}_{q}")
                        for q in range(4)]
                for k in range(4):
                    if prev_h is not None:
                        hTk[k] = emit_tr(k, t - 1)
                    for q in range(4):
                        nc.tensor.matmul(
                            ps_g[q][:BL, :],
                            hTk[k][:],
                            whh_sb[k][:, q * 512:(q + 1) * 512],
                            start=(k == 0), stop=False,
                        )
                mi = (t * BL) // 128
                rsel = (t * BL) % 128
                for q in range(4):
                    nc.tensor.matmul(
                        ps_g[q][:BL, :],
                        ident_r[:, rsel:rsel + BL],
                        xw_sb[mi][:, q * 512:(q + 1) * 512],
                        start=False, stop=True,
                    )
                c_next = [None] * 4
                h_new = [None] * 4
                for q in range(4):
                    act = pw.tile([BL, 512], F32, tag=f"a{q}")
                    nc.scalar.activation(act[:, :384], ps_g[q][:BL, :384], AF.Sigmoid)
                    nc.scalar.activation(
                        act[:, 384:512], ps_g[q][:BL, 384:512], AF.Tanh
                    )
                    fc = pw.tile([BL, 128], F32, tag=f"fc{q}")
                    nc.gpsimd.tensor_tensor(
                        fc[:], act[:, 128:256], c_cur[q][:], ALU.mult
                    )
                    ig = pw.tile([BL, 128], F32, tag=f"ig{q}")
                    nc.vector.tensor_tensor(
                        ig[:], act[:, 0:128], act[:, 384:512], ALU.mult
                    )
                    cn = pw.tile([BL, 128], F32, tag=f"c{q}")
                    nc.gpsimd.tensor_tensor(cn[:], fc[:], ig[:], ALU.add)
                    tcq = pw.tile([BL, 128], F32, tag=f"tc{q}")
                    nc.scalar.activation(tcq[:], cn[:], AF.Tanh)
                    hq = pw.tile([BL, 128], F32, tag=f"h{q}")
                    nc.vector.tensor_tensor(hq[:], act[:, 256:384], tcq[:], ALU.mult)
                    c_next[q] = cn
                    h_new[q] = hq
                    if t == T - 1:
                        nc.sync.dma_start(ho_d[:, q * 128:(q + 1) * 128], hq[:])
                        nc.sync.dma_start(co_d[:, q * 128:(q + 1) * 128], cn[:])
                prev_h = h_new
                c_cur = c_next

                if t == T - 1:
                    for k in range(4):
                        hTk[k] = emit_tr(k, T - 1)
                for m, (moff, msz) in enumerate(chunks):
                    tlast = (moff + msz) // BL - 1
                    if (tlast < T - 1 and t == tlast + 1) or \
                       (tlast == T - 1 and t == T - 1):
                        attn_and_vocab(m, moff, msz)

    nc.compile()
    return nc


_NC_CACHE = {}


def _get_nc(T_=T, V_=V, BL_=BL):
    key = (T_, V_, BL_)
    if key not in _NC_CACHE:
        _NC_CACHE[key] = build_decoder(T_, V_, BL_)
    return _NC_CACHE[key]


def _prep_core(c, x_all, enc, h0, c0, common):
    sl = slice(c * BL, (c + 1) * BL)
    xs = np.ascontiguousarray(x_all[sl])
    e_ = np.ascontiguousarray(enc[sl])
    h0s = np.ascontiguousarray(h0[sl])
    c0s = np.ascontiguousarray(c0[sl])
    d = dict(common)
    d["xt"] = xs.transpose(2, 1, 0).reshape(E, T * BL).astype(np.float32)
    d["enc_l"] = e_.transpose(1, 0, 2).reshape(S, BL * H2).astype(BF16_NP)
    d["enc_t"] = e_.transpose(2, 0, 1).reshape(H2, BL * S).astype(BF16_NP)
    d["h0t"] = (
        h0s.T.reshape(4, 128, BL).transpose(1, 0, 2).reshape(128, 4 * BL)
        .astype(np.float32)
    )
    d["c0"] = c0s.astype(np.float32)
    return d


def kernel(inputs, encoder_outputs, h0, c0, emb, W_ih, W_hh, b_ih, b_hh,
           W_att, W_comb, b_comb, W_out, b_out):
    inputs = np.asarray(inputs)
    enc = np.asarray(encoder_outputs, dtype=np.float32)
    h0 = np.asarray(h0, dtype=np.float32)
    c0 = np.asarray(c0, dtype=np.float32)
    emb = np.asarray(emb, dtype=np.float32)
    W_ih = np.asarray(W_ih, dtype=np.float32)
    W_hh = np.asarray(W_hh, dtype=np.float32)
    b_ih = np.asarray(b_ih, dtype=np.float32)
    b_hh = np.asarray(b_hh, dtype=np.float32)
    W_att = np.asarray(W_att, dtype=np.float32)
    W_comb = np.asarray(W_comb, dtype=np.float32)
    b_comb = np.asarray(b_comb, dtype=np.float32)
    W_out = np.asarray(W_out, dtype=np.float32)
    b_out = np.asarray(b_out, dtype=np.float32)

    x_all = emb[inputs.astype(np.int64)]  # embedding gather (sharding prep)
    perm = gate_perm()

    common = {
        "wih_t": np.ascontiguousarray(W_ih.T[:, perm]).astype(np.float32),
        "whh_t": np.ascontiguousarray(W_hh.T[:, perm]).astype(np.float32),
        "b_ih": np.ascontiguousarray(b_ih[perm]),
        "b_hh": np.ascontiguousarray(b_hh[perm]),
        "watt_t": np.ascontiguousarray(W_att.T).astype(BF16_NP),
        "wcomb_t": np.ascontiguousarray(W_comb.T).astype(BF16_NP),
        "b_comb": b_comb,
        "wo_t": np.ascontiguousarray(W_out.T).astype(BF16_NP),
        "b_out": b_out,
    }
    in_maps = [_prep_core(c, x_all, enc, h0, c0, common) for c in range(NCORES)]

    nc = _get_nc()
    res = run_bass_kernel_spmd(nc, in_maps, core_ids=list(range(NCORES)))
    global LAST_RESULT
    LAST_RESULT = res
    logits = np.concatenate([r["logits"] for r in res.results], axis=0)
    h = np.concatenate([r["h_out"] for r in res.results], axis=0)
    c = np.concatenate([r["c_out"] for r in res.results], axis=0)
    return logits, h, c
